# revision 1
# baseline (speedup 1.0000x reference)
"""Trainium2 Bass kernel for the Clifford (geometric) product on Cl(3,0).

out[n, k] = sum_{i,j} S[i,j,k] * a[n,i] * b[n,j],  S = structure constants
(64 nonzeros, one per (i,j), signs +-1).

Strategy (per NeuronCore, batch sharded 8 ways):
  - Tiles of 128 partitions x E multivectors/partition, natural interleaved
    layout [128, E*8] (contiguous DMA).
  - The 64 signed products are emitted by ~23 DVE ops (tensor_tensor /
    scalar_tensor_tensor) whose access patterns enumerate "affine boxes" of
    (i, j, output-slot) triples; signs are folded into the STT immediate.
  - Products land grouped 8-per-output-component; the 8-way sums run as
    3-level trees, split between the Vector engine (k < KD) and GPSIMD
    (k >= KD) so both engines work in parallel.
"""

import os

# Whole-tile dependency tracking: the ~23 interleaved strided product writes
# per tile otherwise become per-subtile dep edges, whose un-coalesced sem
# waits overflow the ISA's per-instruction wait-command limit.
os.environ.setdefault("BY_DEFAULT_DISABLE_SUBTILE_DEPS", "1")

import numpy as np
from itertools import combinations, permutations

import concourse.bass as bass
import concourse.bacc as bacc
import concourse.mybir as mybir
from concourse import bass_utils
from concourse.tile import TileContext

# ---------------------------------------------------------------- geometry
N_TOTAL = 4194304
N_CORES = 8
NC = N_TOTAL // N_CORES        # 524288 multivectors per core
P = 128                        # partitions
E = 256                        # multivectors per partition per tile
TILE_MV = P * E                # 32768
N_TILES = NC // TILE_MV        # 16
KD = 2                         # components 0..KD-1 reduced on DVE, rest GPSIMD
DMA_CHUNKS = 4                 # partition-range chunks per tensor DMA

F32 = mybir.dt.float32


# ------------------------------------------------- structure constants S
def _build_S():
    basis = [(), (0,), (1,), (2,), (0, 1), (0, 2), (1, 2), (0, 1, 2)]
    b2i = {b: i for i, b in enumerate(basis)}
    S = np.zeros((8, 8, 8), dtype=np.int32)
    for i, a in enumerate(basis):
        for j, b in enumerate(basis):
            comb = list(a) + list(b)
            sign = 1
            n = len(comb)
            for pn in range(n):
                for pos in range(n - 1 - pn):
                    if comb[pos] > comb[pos + 1]:
                        comb[pos], comb[pos + 1] = comb[pos + 1], comb[pos]
                        sign *= -1
            red = []
            idx = 0
            while idx < len(comb):
                if idx + 1 < len(comb) and comb[idx] == comb[idx + 1]:
                    idx += 2
                else:
                    red.append(comb[idx])
                    idx += 1
            S[i, j, b2i[tuple(red)]] = sign
    return S


# ------------------------------------------- affine box cover of the terms
def _box4_assign(tset):
    for split in combinations(range(4), 2):
        g1 = [tset[x] for x in split]
        g2 = [tset[x] for x in range(4) if x not in split]
        for p1 in permutations(g1):
            d1 = (p1[1][0] - p1[0][0], p1[1][1] - p1[0][1])
            for p2 in permutations(g2):
                d2 = (p2[1][0] - p2[0][0], p2[1][1] - p2[0][1])
                if d1 == d2:
                    return [p1[0], p1[1], p2[0], p2[1]]
    return None


def _cover_group(grp):
    best = None

    def rec(rem, acc):
        nonlocal best
        if len(rem) < 4:
            boxes = list(acc)
            r = list(rem)
            while len(r) >= 2:
                boxes.append([r[0], r[1]])
                r = r[2:]
            if r:
                boxes.append([r[0]])
            if best is None or len(boxes) < len(best):
                best = boxes
            return
        found4 = False
        for sub in combinations(range(len(rem)), 4):
            tset = [rem[x] for x in sub]
            a = _box4_assign(tset)
            if a:
                found4 = True
                rec([rem[x] for x in range(len(rem)) if x not in sub], acc + [a])
        if not found4:
            boxes = list(acc)
            r = list(rem)
            while len(r) >= 2:
                boxes.append([r[0], r[1]])
                r = r[2:]
            if r:
                boxes.append([r[0]])
            if best is None or len(boxes) < len(best):
                best = boxes

    rec(grp, [])
    return best


def _gen_ops(kd, split_four=False):
    """Product-op table. Each op: (sign, c1, c2, a_aff, b_aff, slot_aff, region)
    where *_aff = (offset, d1, d0) over a (c1 x c2) beta grid, slot indexes the
    region's product tile ([region-local k] * 8 + rank), region 0 = k<kd (DVE),
    region 1 = k>=kd (GPSIMD)."""
    S = _build_S()
    boxes = []
    for k in range(8):
        for sign in (1, -1):
            grp = [(i, j) for i in range(8) for j in range(8) if S[i, j, k] == sign]
            if not grp:
                continue
            for b in _cover_group(grp):
                boxes.append(dict(sign=sign, pairs=[(k, i, j) for (i, j) in b]))

    def region(k):
        return 0 if k < kd else 1

    # merge 2-boxes with equal (di, dj) deltas, same sign, same region
    twos = [b for b in boxes if len(b["pairs"]) == 2]
    others = [b for b in boxes if len(b["pairs"]) != 2]
    used = [False] * len(twos)
    merged = []
    for x in range(len(twos)):
        if used[x]:
            continue
        bx = twos[x]
        dx = tuple(np.subtract(bx["pairs"][1][1:], bx["pairs"][0][1:]))
        mx = None
        for y in range(x + 1, len(twos)):
            if used[y] or twos[y]["sign"] != bx["sign"]:
                continue
            if region(twos[y]["pairs"][0][0]) != region(bx["pairs"][0][0]):
                continue
            dy = tuple(np.subtract(twos[y]["pairs"][1][1:], twos[y]["pairs"][0][1:]))
            if dx == dy:
                mx = y
                break
        used[x] = True
        if mx is not None:
            used[mx] = True
            merged.append(dict(sign=bx["sign"], pairs=bx["pairs"] + twos[mx]["pairs"]))
        else:
            merged.append(bx)

    final = others + merged
    next_r = {k: 0 for k in range(8)}

    def slot(k, r):
        kk = k if k < kd else k - kd
        return kk * 8 + r

    ops = []
    for b in final:
        prs = b["pairs"]
        n = len(prs)
        if n == 4:
            k_a, k_b = prs[0][0], prs[2][0]
            ra = next_r[k_a]; next_r[k_a] += 2
            rb = next_r[k_b]; next_r[k_b] += 2
            slots = [slot(k_a, ra), slot(k_a, ra + 1), slot(k_b, rb), slot(k_b, rb + 1)]
            c1, c2 = 2, 2
        elif n == 2:
            k_a = prs[0][0]
            ra = next_r[k_a]; next_r[k_a] += 2
            slots = [slot(k_a, ra), slot(k_a, ra + 1)]
            c1, c2 = 1, 2
        else:
            k_a = prs[0][0]
            ra = next_r[k_a]; next_r[k_a] += 1
            slots = [slot(k_a, ra)]
            c1, c2 = 1, 1

        def aff(vals):
            if len(vals) == 1:
                return (vals[0], 0, 0)
            if len(vals) == 2:
                return (vals[0], 0, vals[1] - vals[0])
            o = vals[0]
            d0 = vals[1] - vals[0]
            d1 = vals[2] - vals[0]
            assert vals[3] == o + d0 + d1
            return (o, d1, d0)

        ops.append((
            b["sign"], c1, c2,
            aff([p[1] for p in prs]),
            aff([p[2] for p in prs]),
            aff(slots),
            region(prs[0][0]),
        ))
    assert all(v == 8 for v in next_r.values())
    # The NEFF verifier restricts ScalarTensorTensor (used for sign=-1) to
    # <=3D APs (partition + 2 free dims); split negative 4-boxes into 2-boxes.
    out_ops = []
    for (sign, c1, c2, a, b, s, reg) in ops:
        if (sign == -1 or split_four) and c1 == 2:
            for b1 in range(2):
                out_ops.append((
                    sign, 1, c2,
                    (a[0] + a[1] * b1, 0, a[2]),
                    (b[0] + b[1] * b1, 0, b[2]),
                    (s[0] + s[1] * b1, 0, s[2]),
                    reg,
                ))
        else:
            out_ops.append((sign, c1, c2, a, b, s, reg))
    return out_ops


# ------------------------------------------------------------ bass builder
def _mkap(base, dims, offset):
    """Custom free-dim AP over an SBUF tile AP: dims = [(stride, count), ...]."""
    ap = base.copy()
    part = list(base.ap[0])
    ap.ap = mybir.VecI64Pair([part] + [[d, c] for (d, c) in dims])
    ap.offset = base.offset + offset
    return ap


def build_nc(nc_mv=NC, e=E, kd=KD, reps=1, stages=("prod", "dtree", "gtree"), mode="tree"):
    n_tiles = nc_mv // (P * e)
    assert n_tiles * P * e == nc_mv
    if mode in ("scan", "tilesplit"):
        kd = 8                       # all products in one region/tile
    ops = _gen_ops(kd)
    ops_gps = _gen_ops(kd, split_four=True) if mode == "tilesplit" else None
    kg = 8 - kd                      # gpsimd component count
    w0, w1 = kd * 8, kg * 8          # product-tile slots per mv per region

    nc = bacc.Bacc("TRN2", target_bir_lowering=False, debug=False)
    a_d = nc.dram_tensor("a", [nc_mv, 8], F32, kind="ExternalInput")
    b_d = nc.dram_tensor("b", [nc_mv, 8], F32, kind="ExternalInput")
    o_d = nc.dram_tensor("o", [nc_mv, 8], F32, kind="ExternalOutput")

    a_v = a_d.ap().rearrange("(t p e) c -> t p (e c)", t=n_tiles, p=P)
    b_v = b_d.ap().rearrange("(t p e) c -> t p (e c)", t=n_tiles, p=P)
    o_v = o_d.ap().rearrange("(t p e) c -> t p (e c)", t=n_tiles, p=P)

    mult = mybir.AluOpType.mult
    add = mybir.AluOpType.add

    with TileContext(nc) as tc:
        with (
            tc.tile_pool(name="io", bufs=2) as io_pool,
            tc.tile_pool(name="prod", bufs=2) as prod_pool,
        ):
          def body(_i=None):
            for t in range(n_tiles):
                a_t = io_pool.tile([P, 8 * e], F32, tag="a")
                b_t = io_pool.tile([P, 8 * e], F32, tag="b")
                o_t = io_pool.tile([P, 8 * e], F32, tag="o")
                on_gps = (mode == "tilesplit") and (t % 3 == 2)
                if mode == "tilesplit":
                    tag = "pg" if on_gps else "pd"
                    pd_t = prod_pool.tile([P, w0 * e], F32, tag=tag, bufs=1,
                                          name=f"p_{tag}")
                    pg_t = pd_t
                elif mode == "scan":
                    pd_t = prod_pool.tile([P, w0 * e], F32, tag="pd", bufs=1)
                    pg_t = pd_t  # unused
                    pref_t = prod_pool.tile([P, 1 + 32 * e], F32, tag="pref",
                                            bufs=1)
                else:
                    pd_t = prod_pool.tile([P, w0 * e], F32, tag="pd")
                    if w1 > 0:
                        pg_t = prod_pool.tile([P, w1 * e], F32, tag="pg")
                    else:
                        pg_t = pd_t

                # One dma_start per tensor: a single InstDMACopy is split
                # across all 16 SDMA engines by the runtime, so chunking adds
                # no bandwidth — only extra DMAHW sem waits on consumers.
                nc.sync.dma_start(out=a_t[:, :], in_=a_v[t])
                nc.scalar.dma_start(out=b_t[:, :], in_=b_v[t])

                # ---- products ----
                tile_ops = ops
                if mode == "tilesplit" and on_gps:
                    tile_ops = ops_gps
                for (sign, c1, c2, (ao, ad1, ad0), (bo, bd1, bd0),
                     (so, sd1, sd0), reg) in (tile_ops if "prod" in stages else []):
                    p_t, w = (pd_t, w0) if reg == 0 else (pg_t, w1)
                    if mode == "hybrid":
                        eng = nc.gpsimd if reg == 1 else nc.vector
                    elif mode == "tilesplit":
                        eng = nc.gpsimd if on_gps else nc.vector
                    else:
                        eng = nc.vector
                    dims_a = [(8, e), (ad1, c1), (ad0, c2)]
                    dims_b = [(8, e), (bd1, c1), (bd0, c2)]
                    dims_s = [(w, e), (sd1, c1), (sd0, c2)]
                    in0 = _mkap(a_t, dims_a, ao)
                    in1 = _mkap(b_t, dims_b, bo)
                    out = _mkap(p_t, dims_s, so)
                    if sign == 1:
                        eng.tensor_tensor(out=out, in0=in0, in1=in1, op=mult)
                    else:
                        eng.scalar_tensor_tensor(
                            out=out, in0=in0, scalar=-1.0, in1=in1,
                            op0=mult, op1=mult)

                # ---- reduction trees ----
                def tree(eng, p_t, w, nk, k0):
                    # L1: slots i<4 += i>=4 ; L2: i<2 += i in 2:4 ; L3 -> o_t
                    eng.tensor_tensor(
                        out=_mkap(p_t, [(w, e), (8, nk), (1, 4)], 0),
                        in0=_mkap(p_t, [(w, e), (8, nk), (1, 4)], 0),
                        in1=_mkap(p_t, [(w, e), (8, nk), (1, 4)], 4),
                        op=add)
                    eng.tensor_tensor(
                        out=_mkap(p_t, [(w, e), (8, nk), (1, 2)], 0),
                        in0=_mkap(p_t, [(w, e), (8, nk), (1, 2)], 0),
                        in1=_mkap(p_t, [(w, e), (8, nk), (1, 2)], 2),
                        op=add)
                    eng.tensor_tensor(
                        out=_mkap(o_t, [(8, e), (1, nk)], k0),
                        in0=_mkap(p_t, [(w, e), (8, nk)], 0),
                        in1=_mkap(p_t, [(w, e), (8, nk)], 1),
                        op=add)

                if mode == "scan":
                    if "dtree" in stages:
                        # products+sums in one pass: prefix-sum the product
                        # stream pairwise, then difference every 4th prefix to
                        # extract each 8-product group sum.
                        nc.vector.memset(pref_t[:, 0:1], 0.0)
                        nc.vector.tensor_tensor_scan(
                            out=_mkap(pref_t, [(1, 32 * e)], 1),
                            data0=_mkap(pd_t, [(2, 32 * e)], 0),
                            data1=_mkap(pd_t, [(2, 32 * e)], 1),
                            initial=0.0,
                            op0=mybir.AluOpType.add,
                            op1=mybir.AluOpType.add)
                        nc.vector.tensor_tensor(
                            out=_mkap(o_t, [(1, 8 * e)], 0),
                            in0=_mkap(pref_t, [(4, 8 * e)], 4),
                            in1=_mkap(pref_t, [(4, 8 * e)], 0),
                            op=mybir.AluOpType.subtract)
                    else:
                        nc.vector.memset(o_t[:, 0:1], 0.0)
                elif mode == "tilesplit":
                    tree(nc.gpsimd if on_gps else nc.vector, pd_t, w0, 8, 0)
                elif "dtree" in stages or "gtree" in stages:
                    if "dtree" in stages:
                        tree(nc.vector, pd_t, w0, kd, 0)
                    if "gtree" in stages and kg > 0:
                        tree(nc.gpsimd, pg_t, w1, kg, kd)
                else:
                    # ablation builds: satisfy read-before-write tracking
                    nc.vector.memset(o_t[:, 0:1], 0.0)

                nc.sync.dma_start(out=o_v[t], in_=o_t[:, :])
          if reps == 1:
            body()
          else:
            with tc.For_i(0, reps, 1) as _i:
                body(_i)
    nc.compile()
    return nc


_NC_CACHE = {}


def _get_nc(nc_mv, e, kd):
    key = (nc_mv, e, kd)
    if key not in _NC_CACHE:
        _NC_CACHE[key] = build_nc(nc_mv, e, kd)
    return _NC_CACHE[key]


def kernel(a, b, M=None, **_):
    a = np.ascontiguousarray(np.asarray(a, dtype=np.float32))
    b = np.ascontiguousarray(np.asarray(b, dtype=np.float32))
    n = a.shape[0]
    assert n % N_CORES == 0
    nc_mv = n // N_CORES
    nc = _get_nc(nc_mv, E, KD)
    a_sh = a.reshape(N_CORES, nc_mv, 8)
    b_sh = b.reshape(N_CORES, nc_mv, 8)
    in_maps = [{"a": a_sh[c], "b": b_sh[c]} for c in range(N_CORES)]
    res = bass_utils.run_bass_kernel_spmd(nc, in_maps, core_ids=list(range(N_CORES)))
    out = np.concatenate([r["o"].reshape(nc_mv, 8) for r in res.results], axis=0)
    return out



# revision 2
# speedup vs baseline: 2.3329x; 2.3329x over previous
"""Trainium2 Bass kernel for the Clifford (geometric) product on Cl(3,0), v2.

Strategy (per NeuronCore, batch sharded 8 ways):
  Cl(3,0) ~= Mat2(C) via the Pauli representation. The product becomes a
  2x2 complex matrix multiply: 32 real multiplies + 48 adds per sample,
  vs 64 multiplies + 56 adds for the direct structure-constant form.

  Layout: per-partition planar bf16. ScalarE deinterleaves the DMA'd
  fp32 interleaved tiles into bf16 component planes (folding the 1/2
  scale of the forward transform into the b cast). All compute runs on
  the Vector engine as step-1 bf16 tensor_tensor ops -> 2x_1P perf mode
  (2 elem/cyc); measured per-op durations match the mode exactly.
  GPSIMD is deliberately idle: it shares an SBUF port with the DVE, and
  every attempt to offload work there slowed the DVE more than it
  helped. ScalarE re-interleaves the bf16 planar output to fp32 for a
  plain HWDGE store; the re-interleave of tile t is emitted during tile
  t+1 so no engine queue blocks on a late-pipeline stage.

  Plane orders:
    A[p,r,s]: idx 4p+2r+s (s=0:Re,1:Im) of matrix entry M[p][r]
    B[r,q,s]: idx 4r+2q+s of N[r][q]
    pc slot = 16p + 8q + 4*(sM^sN) + 2*sM + r   (products M[p,r]*N[r,q])
    c[o]: idx 4p+2q+ri of C[p][q] (ri=0:Re,1:Im)
"""

import os

os.environ.setdefault("BY_DEFAULT_DISABLE_SUBTILE_DEPS", "1")

import numpy as np

N_TOTAL = 4194304
N_CORES = 8
NC = N_TOTAL // N_CORES
P = 128

F32 = None
BF16 = None


def _dt():
    global F32, BF16
    import concourse.mybir as mybir

    F32 = mybir.dt.float32
    BF16 = mybir.dt.bfloat16


# --------------------------------------------------------------- op tables
# spec = (tile, offset_elems, dims) with dims [(stride, count), ...] in elems,
# given e = samples per plane. Tiles: a32i,b32i,o32i fp32 [128, 8e];
# a16p,b16p,A,B,c bf16 [128, 8e]; pc bf16 [128, 32e].


def gen_ops(e, split_q=False, l1_split=8, tfA_src="a32i"):
    """Return list of (group, alu, out_spec, in0_spec, in1_spec).

    c-planes live in pc slots 4*o+1 (free after L1, which only writes ranks
    0/2); the inverse transform writes bf16 planar o16p, re-interleaved to
    fp32 by ScalarE. l1_split: number of L1 bases (of 8) in the first op
    (group L1a); the rest go in L1b (assignable to another engine).
    tfA_src="a32i": the A-transform reads the fp32 interleaved tile
    directly (component c at elem offset c, sample stride 8) — no a16p
    deinterleave; the 1/2 scale folds into the b deinterleave instead.
    """
    ops = []

    def pl(t, off, dims):  # offsets/strides in plane units -> elems
        return (t, off * e, [(s * e, c) for (s, c) in dims[:-1]] + [dims[-1]])

    def il(t, off, dims):  # interleaved: comp offset/strides raw, samples x8
        return (t, off, [(s, c) for (s, c) in dims[:-1]] + [(8, dims[-1][1])])

    # transforms: (out two planes), (in0 planes), (in1 planes)
    tr = [
        ("add", (0, 1), (0, 4), (3, 4)),
        ("add", (4, 1), (1, 1), (5, 1)),
        ("sub", (2, 1), (1, 5), (5, -3)),
        ("sub", (6, 1), (0, 7), (3, 1)),
    ]
    srcA = (tfA_src, il) if tfA_src == "a32i" else (tfA_src, pl)
    for dst, (src, f) in (("A", srcA), ("B", ("b16p", pl))):
        for alu, (oo, od), (i0, d0), (i1, d1) in tr:
            ops.append((
                f"tf{dst}", alu,
                pl(dst, oo, [(od, 2), (1, e)]),
                f(src, i0, [(d0, 2), (1, e)]),
                f(src, i1, [(d1, 2), (1, e)]),
            ))

    # products: (p, sM, sN) over (q, r) grid
    for p in range(2):
        for sM in range(2):
            for sN in range(2):
                oo = 16 * p + 4 * (sM ^ sN) + 2 * sM
                if split_q:
                    for q in range(2):
                        ops.append((
                            "prod", "mult",
                            pl("pc", oo + 8 * q, [(1, 2), (1, e)]),
                            pl("A", 4 * p + sM, [(2, 2), (1, e)]),
                            pl("B", sN + 2 * q, [(4, 2), (1, e)]),
                        ))
                else:
                    ops.append((
                        "prod", "mult",
                        pl("pc", oo, [(8, 2), (1, 2), (1, e)]),
                        pl("A", 4 * p + sM, [(0, 2), (2, 2), (1, e)]),
                        pl("B", sN, [(2, 2), (4, 2), (1, e)]),
                    ))

    # L1: slots base+2t += base+2t+1 over 8 bases, split l1_split/(8-l1_split)
    for grp, b0, nb in (("L1a", 0, l1_split), ("L1b", l1_split, 8 - l1_split)):
        if nb <= 0:
            continue
        ops.append((
            grp, "add",
            pl("pc", 4 * b0, [(4, nb), (2, 2), (1, e)]),
            pl("pc", 4 * b0, [(4, nb), (2, 2), (1, e)]),
            pl("pc", 4 * b0 + 1, [(4, nb), (2, 2), (1, e)]),
        ))

    # L2 into c-planes at pc slot 4*o+1: c[Re:o=2(2p+q)] = rank0 - rank2 ;
    # c[Im] = rank0(+4) + rank2(+6)
    ops.append((
        "L2s", "sub",
        pl("pc", 1, [(16, 2), (8, 2), (1, e)]),
        pl("pc", 0, [(16, 2), (8, 2), (1, e)]),
        pl("pc", 2, [(16, 2), (8, 2), (1, e)]),
    ))
    ops.append((
        "L2a", "add",
        pl("pc", 5, [(16, 2), (8, 2), (1, e)]),
        pl("pc", 4, [(16, 2), (8, 2), (1, e)]),
        pl("pc", 6, [(16, 2), (8, 2), (1, e)]),
    ))

    # inverse: bf16 planar o16p; c-plane o at pc slot 4*o+1
    inv = [
        ("add", (0, 7), (0, 1), (6, 1)),
        ("add", (1, 5), (2, 1), (4, 1)),
        ("sub", (3, 1), (0, 1), (6, 1)),
        ("sub", (5, -3), (4, 1), (2, 1)),
    ]
    for alu, (oo, od), (i0, d0), (i1, d1) in inv:
        ops.append((
            "inv", alu,
            pl("o16p", oo, [(od, 2), (1, e)]),
            pl("pc", 4 * i0 + 1, [(4 * d0, 2), (1, e)]),
            pl("pc", 4 * i1 + 1, [(4 * d1, 2), (1, e)]),
        ))
    return ops


# ------------------------------------------------------- numpy validation
def _walk(buf, spec):
    t, off, dims = spec
    idx = np.zeros([c for (_, c) in dims], dtype=np.int64) + off
    for d, (s, c) in enumerate(dims):
        sh = [1] * len(dims)
        sh[d] = c
        idx = idx + (np.arange(c) * s).reshape(sh)
    return buf[t], idx


def simulate(a, b, e):
    """Run the op tables in numpy (fp32, per partition-lane) for validation."""
    n = a.shape[0]
    assert n == e
    bufs = {
        "a32i": (a.reshape(-1) * 1.0).astype(np.float32),
        "b32i": b.reshape(-1).astype(np.float32),
        "a16p": np.zeros(8 * e, np.float32),
        "b16p": np.zeros(8 * e, np.float32),
        "A": np.zeros(8 * e, np.float32),
        "B": np.zeros(8 * e, np.float32),
        "pc": np.zeros(32 * e, np.float32),
        "o16p": np.zeros(8 * e, np.float32),
        "o32i": np.zeros(8 * e, np.float32),
    }
    # deinterleave (+0.5 scale on b; a read raw by tfA from a32i)
    for c_ in range(8):
        bufs["a16p"][c_ * e:(c_ + 1) * e] = bufs["a32i"][c_::8]
        bufs["b16p"][c_ * e:(c_ + 1) * e] = 0.5 * bufs["b32i"][c_::8]
    alu = {"add": np.add, "sub": np.subtract, "mult": np.multiply}
    for (_, op, o, i0, i1) in gen_ops(e):
        ob, oi = _walk(bufs, o)
        b0, x0 = _walk(bufs, i0)
        b1, x1 = _walk(bufs, i1)
        ob[oi] = alu[op](b0[x0], b1[x1])
    # re-interleave
    for c_ in range(8):
        bufs["o32i"][c_::8] = bufs["o16p"][c_ * e:(c_ + 1) * e]
    return bufs["o32i"].reshape(e, 8)


# ------------------------------------------------------------ bass builder
def _mkap(base, dims, offset):
    import concourse.mybir as mybir

    ap = base.copy()
    part = list(base.ap[0])
    ap.ap = mybir.VecI64Pair([part] + [[d, c] for (d, c) in dims])
    ap.offset = base.offset + offset
    return ap


# op-group -> engine name ("vector" | "gpsimd"); "deint" also allows "scalar"
DEFAULT_ASSIGN = {
    "deint": "scalar",
    "tfA": "vector",
    "tfB": "vector",
    "prod": "vector",
    "L1a": "vector",
    "L1b": "vector",
    "L2s": "vector",
    "L2a": "vector",
    "inv": "vector",
}


def build_nc(nc_mv=NC, e=256, assign=None, split_q=False, l1_split=8,
             tfA_src="a16p", reint_gps=0, nbufs=3):
    import concourse.bacc as bacc
    import concourse.mybir as mybir
    from concourse.tile import TileContext

    _dt()
    assign = dict(DEFAULT_ASSIGN, **(assign or {}))
    n_tiles = nc_mv // (P * e)
    assert n_tiles * P * e == nc_mv
    ops = gen_ops(e, split_q=split_q, l1_split=l1_split, tfA_src=tfA_src)

    nc = bacc.Bacc("TRN2", target_bir_lowering=False, debug=False)
    a_d = nc.dram_tensor("a", [nc_mv, 8], F32, kind="ExternalInput")
    b_d = nc.dram_tensor("b", [nc_mv, 8], F32, kind="ExternalInput")
    o_d = nc.dram_tensor("o", [nc_mv, 8], F32, kind="ExternalOutput")

    a_v = a_d.ap().rearrange("(t p e) c -> t p (e c)", t=n_tiles, p=P)
    b_v = b_d.ap().rearrange("(t p e) c -> t p (e c)", t=n_tiles, p=P)
    o_v = o_d.ap().rearrange("(t p e) c -> t p (e c)", t=n_tiles, p=P)

    ALU = {
        "add": mybir.AluOpType.add,
        "sub": mybir.AluOpType.subtract,
        "mult": mybir.AluOpType.mult,
    }

    with TileContext(nc) as tc:
        with (
            tc.tile_pool(name="io", bufs=nbufs) as io_pool,
            tc.tile_pool(name="pln", bufs=nbufs) as pln_pool,
        ):
            def emit_reint(o16p, o32i, t):
                # re-interleave + cast to fp32, split ScalarE / GPSIMD
                ng = reint_gps
                if ng < 8:
                    nc.scalar.copy(
                        _mkap(o32i, [(1, 8 - ng), (8, e)], 0),
                        _mkap(o16p, [(e, 8 - ng), (1, e)], 0),
                    )
                if ng > 0:
                    nc.gpsimd.tensor_scalar_mul(
                        _mkap(o32i, [(1, ng), (8, e)], 8 - ng),
                        _mkap(o16p, [(e, ng), (1, e)], (8 - ng) * e),
                        1.0,
                    )
                nc.sync.dma_start(out=o_v[t], in_=o32i[:, :])

            pending = None
            for t in range(n_tiles):
                a32i = io_pool.tile([P, 8 * e], F32, tag="a32i")
                b32i = io_pool.tile([P, 8 * e], F32, tag="b32i")
                o32i = io_pool.tile([P, 8 * e], F32, tag="o32i")
                b16p = pln_pool.tile([P, 8 * e], BF16, tag="b16p")
                A_t = pln_pool.tile([P, 8 * e], BF16, tag="A")
                B_t = pln_pool.tile([P, 8 * e], BF16, tag="B")
                o16p = pln_pool.tile([P, 8 * e], BF16, tag="o16p")
                pc_t = pln_pool.tile([P, 32 * e], BF16, tag="pc")

                tiles = {
                    "a32i": a32i, "b32i": b32i, "o32i": o32i,
                    "b16p": b16p, "A": A_t, "B": B_t,
                    "o16p": o16p, "pc": pc_t,
                }
                if tfA_src == "a16p":
                    a16p = pln_pool.tile([P, 8 * e], BF16, tag="a16p")
                    tiles["a16p"] = a16p

                nc.sync.dma_start(out=a32i[:, :], in_=a_v[t])
                nc.scalar.dma_start(out=b32i[:, :], in_=b_v[t])

                # deinterleave + cast; the 1/2 transform scale folds into b.
                if tfA_src == "a16p":
                    nc.scalar.copy(
                        _mkap(tiles["a16p"], [(e, 8), (1, e)], 0),
                        _mkap(a32i, [(1, 8), (8, e)], 0),
                    )
                nc.scalar.mul(
                    _mkap(b16p, [(e, 8), (1, e)], 0),
                    _mkap(b32i, [(1, 8), (8, e)], 0),
                    0.5,
                )

                for (grp, op, o, i0, i1) in ops:
                    eng = nc.vector if assign[grp] == "vector" else nc.gpsimd
                    to, oo, od = o
                    t0, f0, d0 = i0
                    t1, f1, d1 = i1
                    eng.tensor_tensor(
                        out=_mkap(tiles[to], od, oo),
                        in0=_mkap(tiles[t0], d0, f0),
                        in1=_mkap(tiles[t1], d1, f1),
                        op=ALU[op],
                    )

                # stagger: emit previous tile's re-interleave+store now, so
                # no engine queue blocks on this tile's late stages.
                if pending is not None:
                    emit_reint(*pending)
                pending = (o16p, o32i, t)
            emit_reint(*pending)
    nc.compile()
    return nc


_NC_CACHE = {}


def _get_nc(nc_mv, e=256, **kw):
    key = (nc_mv, e, tuple(sorted(kw.items())))
    if key not in _NC_CACHE:
        _NC_CACHE[key] = build_nc(nc_mv, e, **kw)
    return _NC_CACHE[key]


def kernel(a, b, M=None, **_):
    from concourse import bass_utils

    a = np.ascontiguousarray(np.asarray(a, dtype=np.float32))
    b = np.ascontiguousarray(np.asarray(b, dtype=np.float32))
    n = a.shape[0]
    assert n % N_CORES == 0
    nc_mv = n // N_CORES
    nc = _get_nc(nc_mv)
    a_sh = a.reshape(N_CORES, nc_mv, 8)
    b_sh = b.reshape(N_CORES, nc_mv, 8)
    in_maps = [{"a": a_sh[c], "b": b_sh[c]} for c in range(N_CORES)]
    res = bass_utils.run_bass_kernel_spmd(nc, in_maps, core_ids=list(range(N_CORES)))
    out = np.concatenate([r["o"].reshape(nc_mv, 8) for r in res.results], axis=0)
    return out
